# revision 1
# baseline (speedup 1.0000x reference)
"""Bahdanau-attention Trainium2 kernel (self-contained).

kernel(**inputs) takes the FULL unsharded inputs (as produced by the
problem's setup_inputs) and returns (attn_out, attn_weight) matching the
reference:

  qh = queries @ Wq + bq ; kh = keys @ Wk + bk
  h = tanh(qh[:,:,None,:] + kh[:,None,:,:])
  scores = h @ Wv + bv + cum_subs_masks * -1e9
  attn_weight = softmax(scores, axis over keys)   # (B, NQ, K, 1)
  attn_out = sum_k attn_weight * keys             # (B, NQ, key_size)

Distribution: pure data parallel over the batch dim (256) across 8
NeuronCores (32 batches each); projection weights replicated.

Device mapping per core (pairs of batches per iteration):
  - pre[h, (b,q,k)] = qh + kh built by two accumulating TensorE matmuls per
    512-col PSUM chunk with stride-0 (broadcast) rhs access patterns, fed by
    host-pretransposed queriesT/keysT; b-pairs row-packed on PE partitions
    0:64 / 64:128.  Per-q stride padded to 256 so chunks align with banks.
  - tanh on ScalarE (bias = bq+bk), big strided instructions.
  - scores via per-q matmuls with lhsT = Wv placed in column q, accumulating
    into a (16, 400) PSUM region on top of the mask (identity matmul,
    host-prescaled by -1e9).  float32r operands for full PE rate.
  - softmax without max-subtraction (|scores| <= ||Wv||_1, masked lanes
    underflow to exactly 0 like the reference).
  - w^T via VectorE 32x32 block transposes; attn_out matmuls contract over
    keys; outputs DMA'd per batch.
"""
import sys
sys.path.insert(0, '/opt/trn_rl_repo')
import numpy as np
import concourse.bass as bass
import concourse.tile as tile
from concourse import bacc, mybir

F32 = mybir.dt.float32
F32R = mybir.dt.float32r
AF = mybir.ActivationFunctionType

NQ, K, KD, QD, H = 16, 200, 64, 64, 128
KP = 256                      # padded per-q stride in PSUM
CHUNKS = [(0, 6), (6, 6), (12, 4)]   # (q0, nq) chunks per batch
N_CORES = 8


def _r(ap):
    """View an f32 AP as float32r for full-rate PE matmuls."""
    return ap.bitcast(F32R)


def build_module(NB):
    """Build the per-core Bass module processing NB batches (NB even)."""
    assert NB % 2 == 0
    nc = bacc.Bacc("TRN2", target_bir_lowering=False, debug=False)

    qT_d = nc.dram_tensor("qT", (NB, QD, NQ), F32, kind="ExternalInput").ap()
    kT_d = nc.dram_tensor("kT", (NB, KD, KP), F32, kind="ExternalInput").ap()
    ko_d = nc.dram_tensor("ko", (NB, K, KD), F32, kind="ExternalInput").ap()
    mk_d = nc.dram_tensor("mk", (NB, NQ, K), F32, kind="ExternalInput").ap()
    wq2_d = nc.dram_tensor("wq2", (128, H), F32, kind="ExternalInput").ap()
    wk2_d = nc.dram_tensor("wk2", (128, H), F32, kind="ExternalInput").ap()
    wve_d = nc.dram_tensor("wve", (H, NQ * NQ), F32, kind="ExternalInput").ap()
    i16_d = nc.dram_tensor("i16", (NQ, NQ), F32, kind="ExternalInput").ap()
    bqk_d = nc.dram_tensor("bqk", (H, 1), F32, kind="ExternalInput").ap()

    ao_d = nc.dram_tensor("attn_out", (NB, NQ, KD), F32,
                          kind="ExternalOutput").ap()
    aw_d = nc.dram_tensor("attn_w", (NB, NQ, K), F32,
                          kind="ExternalOutput").ap()

    with tile.TileContext(nc) as tc:
        with (
            tc.tile_pool(name="const", bufs=1) as cpool,
            tc.tile_pool(name="io", bufs=3) as io,
            tc.tile_pool(name="work", bufs=2) as work,
            tc.tile_pool(name="tanh_ps", bufs=2,
                         space=bass.MemorySpace.PSUM) as tps,
            tc.tile_pool(name="misc_ps", bufs=2,
                         space=bass.MemorySpace.PSUM) as mps,
        ):
            wq2_t = cpool.tile([128, H], F32)
            wk2_t = cpool.tile([128, H], F32)
            wve_t = cpool.tile([H, NQ * NQ], F32)
            i16_t = cpool.tile([NQ, NQ], F32)
            bqk_t = cpool.tile([H, 1], F32)
            nc.sync.dma_start(wq2_t[:], wq2_d)
            nc.sync.dma_start(wk2_t[:], wk2_d)
            nc.sync.dma_start(wve_t[:], wve_d)
            nc.sync.dma_start(i16_t[:], i16_d)
            nc.sync.dma_start(bqk_t[:], bqk_d)

            # persistent pair-parity w tiles (rows 16:32 / pad cols are read
            # by the block transposes -> must be initialized once)
            wnorm_p = []
            for i in range(2):
                t = cpool.tile([32, 448], F32, name=f"wnorm{i}")
                nc.vector.memset(t[:], 0.0)
                wnorm_p.append(t)

            for p in range(NB // 2):
                b0 = 2 * p
                # ---- pair input DMAs ----
                qT_t = io.tile([128, NQ], F32)
                kT_t = io.tile([128, KP], F32)
                ko0_t = io.tile([128, 128], F32)
                ko1_t = io.tile([128, 128], F32)
                mk_t = io.tile([NQ, 2 * K], F32)
                nc.sync.dma_start(
                    qT_t[:], qT_d[b0:b0 + 2].rearrange("b d q -> (b d) q"))
                nc.sync.dma_start(
                    kT_t[:], kT_d[b0:b0 + 2].rearrange("b d k -> (b d) k"))
                for bi, kt in enumerate([ko0_t, ko1_t]):
                    nc.sync.dma_start(kt[:, 0:KD], ko_d[b0 + bi, 0:128, :])
                    nc.sync.dma_start(kt[0:K - 128, KD:2 * KD],
                                      ko_d[b0 + bi, 128:K, :])
                nc.sync.dma_start(mk_t[:, 0:K], mk_d[b0])
                nc.sync.dma_start(mk_t[:, K:2 * K], mk_d[b0 + 1])

                tanh_pair = work.tile([128, 2 * NQ * K], F32,
                                      name=f"tanhp_{p}", tag="tanhp")

                # ---- pre = qh + kh via PE, tanh via ACT ----
                for ci, (q0, nq) in enumerate(CHUNKS):
                    tpa = tps.tile([128, 6 * KP], F32, name=f"tp0_{p}_{ci}",
                                   tag="tp")
                    tpb = tps.tile([128, 6 * KP], F32, name=f"tp1_{p}_{ci}",
                                   tag="tp")
                    tp_b = [tpa, tpb]
                    for bi in range(2):
                        pp = 64 * bi
                        tp = tp_b[bi]
                        for j in range(nq // 2):
                            qa = q0 + 2 * j
                            out_chunk = tp[:, j * 2 * KP:(j + 1) * 2 * KP]
                            rhs_q = (qT_t[pp:pp + 64, qa:qa + 2]
                                     .unsqueeze(2).to_broadcast((64, 2, KP)))
                            rhs_k = (kT_t[pp:pp + 64, :]
                                     .unsqueeze(1).to_broadcast((64, 2, KP)))
                            nc.tensor.matmul(out_chunk,
                                             _r(wq2_t[pp:pp + 64, :]),
                                             _r(rhs_q), start=True, stop=False)
                            nc.tensor.matmul(out_chunk,
                                             _r(wk2_t[pp:pp + 64, :]),
                                             _r(rhs_k), start=False, stop=True)
                    for bi in range(2):
                        tp = tp_b[bi]
                        in_ap = (tp[:].rearrange("p (q k) -> p q k", k=KP)
                                 [:, 0:nq, 0:K])
                        out_ap = (tanh_pair[:].rearrange(
                            "p (b q k) -> p b q k", b=2, k=K)
                            [:, bi, q0:q0 + nq, :])
                        nc.scalar.activation(out_ap, in_ap, AF.Tanh,
                                             bias=bqk_t[:])

                # ---- scores (+mask) into misc psum ----
                mp = mps.tile([128, 512], F32, name=f"mp_{p}", tag="mp")
                nc.tensor.matmul(mp[0:NQ, 0:2 * K], _r(i16_t[:]), _r(mk_t[:]),
                                 start=True, stop=False, skip_group_check=True)
                sc_out = (mp[0:NQ, 0:2 * K]
                          .rearrange("p (b k) -> p b k", k=K))
                tanh_q = tanh_pair[:].rearrange("p (b q k) -> p b q k",
                                                b=2, k=K)
                for q in range(NQ):
                    nc.tensor.matmul(
                        sc_out,
                        _r(wve_t[:, q * NQ:(q + 1) * NQ]),
                        _r(tanh_q[:, :, q, :]),
                        start=False, stop=(q == NQ - 1),
                        skip_group_check=True)

                # ---- softmax (no max subtraction) ----
                wraw = work.tile([NQ, 448], F32, name=f"wraw_{p}", tag="wraw")
                wnorm = wnorm_p[p % 2]
                exp_out = wraw[:].rearrange("p (b k) -> p b k",
                                            k=224)[:, :, 0:K]
                exp_in = mp[0:NQ, 0:2 * K].rearrange("p (b k) -> p b k", k=K)
                nc.scalar.activation(exp_out, exp_in, AF.Exp)

                sums = work.tile([NQ, 4], F32, name=f"sums_{p}", tag="sums")
                nc.vector.tensor_reduce(sums[:, 0:2], exp_out,
                                        axis=mybir.AxisListType.X,
                                        op=mybir.AluOpType.add)
                nc.vector.reciprocal(sums[:, 2:4], sums[:, 0:2])
                norm_out = (wnorm[0:NQ, :]
                            .rearrange("p (b k) -> p b k", k=224)[:, :, 0:K])
                recip_b = sums[:, 2:4].unsqueeze(2).to_broadcast((NQ, 2, K))
                nc.vector.tensor_mul(norm_out, exp_out, recip_b)

                # ---- w^T via DVE 32x32 block transposes ----
                wT_hi = [work.tile([128, 32], F32, name=f"wT_hi{i}_{p}",
                                   tag=f"wT_hi{i}") for i in range(2)]
                wT_lo = [work.tile([96, 32], F32, name=f"wT_lo{i}_{p}",
                                   tag=f"wT_lo{i}") for i in range(2)]
                for bi in range(2):
                    for j in range(7):
                        src = wnorm[0:32, bi * 224 + 32 * j:
                                    bi * 224 + 32 * j + 32]
                        if j < 4:
                            dst = wT_hi[bi][32 * j:32 * j + 32, :]
                        else:
                            dst = wT_lo[bi][32 * (j - 4):32 * (j - 4) + 32, :]
                        nc.vector.transpose(dst, src)

                # ---- attn_out matmuls + copy out ----
                attn_sb = work.tile([128, 128], F32, name=f"attn_sb_{p}",
                                    tag="attn_sb")
                for bi in range(2):
                    pp = 64 if bi == 0 else 32
                    c0 = 384 if bi == 0 else 448
                    out_ap = mp[pp:pp + NQ, c0:c0 + KD]
                    kt = [ko0_t, ko1_t][bi]
                    nc.tensor.matmul(out_ap, _r(wT_hi[bi][:, 0:NQ]),
                                     _r(kt[:, 0:KD]),
                                     start=True, stop=False,
                                     skip_group_check=True)
                    nc.tensor.matmul(out_ap, _r(wT_lo[bi][0:K - 128, 0:NQ]),
                                     _r(kt[0:K - 128, KD:2 * KD]),
                                     start=False, stop=True,
                                     skip_group_check=True)
                    nc.vector.tensor_copy(
                        attn_sb[pp:pp + NQ, KD * bi:KD * bi + KD], out_ap)

                # ---- output DMAs ----
                for bi in range(2):
                    pp = 64 if bi == 0 else 32
                    nc.sync.dma_start(
                        ao_d[b0 + bi],
                        attn_sb[pp:pp + NQ, KD * bi:KD * bi + KD])
                    nc.sync.dma_start(
                        aw_d[b0 + bi],
                        wnorm[0:NQ, bi * 224:bi * 224 + K])

    nc.compile()
    return nc


def host_prep(queries, keys, cum_subs_masks, Wq, bq, Wk, bk, Wv, bv,
              n_cores=N_CORES):
    """Full inputs -> per-core in_maps (list of dicts)."""
    B = queries.shape[0]
    NBc = B // n_cores
    queries = np.asarray(queries, dtype=np.float32)
    keys = np.asarray(keys, dtype=np.float32)
    mask = np.asarray(cum_subs_masks, dtype=np.float32).reshape(B, NQ, K)

    qT = np.ascontiguousarray(queries.transpose(0, 2, 1))
    kT = np.zeros((B, KD, KP), dtype=np.float32)
    kT[:, :, 0:K] = keys.transpose(0, 2, 1)
    mk = np.ascontiguousarray(mask * np.float32(-1.0e9))

    Wq = np.asarray(Wq, np.float32); Wk = np.asarray(Wk, np.float32)
    Wv = np.asarray(Wv, np.float32).reshape(H, 1)
    bq = np.asarray(bq, np.float32); bk = np.asarray(bk, np.float32)
    wq2 = np.concatenate([Wq, Wq], axis=0)
    wk2 = np.concatenate([Wk, Wk], axis=0)
    wve = np.zeros((H, NQ * NQ), np.float32)
    for q in range(NQ):
        wve[:, q * NQ + q] = Wv[:, 0]
    i16 = np.eye(NQ, dtype=np.float32)
    bqk = (bq + bk).reshape(H, 1).astype(np.float32)

    in_maps = []
    for c in range(n_cores):
        s = slice(c * NBc, (c + 1) * NBc)
        in_maps.append({
            "qT": np.ascontiguousarray(qT[s]),
            "kT": np.ascontiguousarray(kT[s]),
            "ko": np.ascontiguousarray(keys[s]),
            "mk": np.ascontiguousarray(mk[s]),
            "wq2": wq2, "wk2": wk2, "wve": wve, "i16": i16, "bqk": bqk,
        })
    return in_maps


_MODULE_CACHE = {}


def kernel(queries, keys, cum_subs_masks, num_neg=None, Wq=None, bq=None,
           Wk=None, bk=None, Wv=None, bv=None, **_ignored):
    """Full-input entry point; returns (attn_out, attn_weight)."""
    from concourse import bass_utils

    B = np.asarray(queries).shape[0]
    NB = B // N_CORES
    if NB not in _MODULE_CACHE:
        _MODULE_CACHE[NB] = build_module(NB)
    nc = _MODULE_CACHE[NB]

    in_maps = host_prep(queries, keys, cum_subs_masks, Wq, bq, Wk, bk, Wv, bv)
    res = bass_utils.run_bass_kernel_spmd(
        nc, in_maps, core_ids=list(range(N_CORES)), trace=False)
    ao = np.concatenate([r["attn_out"] for r in res.results], axis=0)
    aw = np.concatenate([r["attn_w"] for r in res.results], axis=0)
    return ao, aw[..., None]
